# revision 4
# baseline (speedup 1.0000x reference)
"""ConvergedInhibition TRN2 kernel (fp8 correction-matmul version).

The reference computes, per pixel (n,h,w), an FFT deconvolution along the
channel axis: y = ifft(fft(x)/fft(k)).real. Since k is fixed, this is a
circular convolution with g = ifft(1/fft(k)): a dense CxC circulant matmul
applied to every pixel, data-parallel over 32 images across 8 cores.

This version exploits the structure y = x + c where c = (G - I) x is a small
correction (||c|| ~ 0.14 ||y||): the device computes only the correction from
fp8(e4m3)-quantized activations and stores it as fp8, halving HBM traffic in
both directions (the DMA roofline). The exact fp32 identity term is added
back on the host during unsharding, so quantization noise only enters scaled
by the correction magnitude (measured total rel err ~8e-3 vs 2e-2 budget).

Rotated frame: z[r] = y[(r+ROT) mod C] aligns the deconv impulse response h
(one-sided, support ~[0,224)) to the diagonal. Keeping chunk distances
d=(zc-jc) mod 4 in {0,1} covers t in [0, 128+q] per output row q (trunc err
~2e-3). For zc>=1 the two kept input chunks are adjacent in SBUF, so each
output tile is ONE fp8 DoubleRow matmul (K=256 at 2x PE rate, 392cyc). zc=0
wraps (jc=3,0) and uses two plain fp8 matmuls instead.

Engine layout (per core): gpsimd issues gt then the 16 act loads on the
SWDGE ring (FIFO keeps gt ahead of the big loads); sync issues the 32 output
stores; vector and scalar alternate 4-tile (1568-col) PSUM->fp8 quad-drains;
tensor runs 160 matmuls at a measured 166ns/tile (LDWEIGHTS prefetch
overlaps matmuls via the PE reorder window). PSUM is one 8-bank tensor; the
drain of quad Q gates tensor's reuse of those 4 banks at quad Q+2.
"""

import numpy as np
import ml_dtypes

import concourse.bass as bass  # noqa: F401  (registers bass types)
import concourse.mybir as mybir
from concourse import bacc
from concourse.bass_utils import run_bass_kernel_spmd

N_CORES = 8
N, C, H, W = 32, 512, 56, 56
HW = H * W                      # 3136
IMGS = N // N_CORES             # 4 images per core
P = 128                         # partitions
NCHUNK = C // P                 # 4
PT = 392                        # pixel tile (free dim), 3136 = 8*392
NPT = HW // PT                  # 8
ROT = 288                       # rotation aligning h's one-sided support
IO_DT = mybir.dt.float8e4
IO_NP = ml_dtypes.float8_e4m3   # matches TRN FP8_EXP4 semantics
N_WARMUP = 14                   # HAM clock-gate warmup matmuls
QT = 4                          # tiles per drain quad
NQ = IMGS * NCHUNK * NPT // QT  # 32 quads

_CACHE = {}


def _build_nc():
    """Raw bacc engine programs with explicit semaphores."""
    nc = bacc.Bacc("TRN2", target_bir_lowering=False, debug=False,
                   num_devices=N_CORES)
    act = nc.dram_tensor("act", [IMGS, C, HW], IO_DT, kind="ExternalInput")
    gt = nc.dram_tensor("gt", [C, C], IO_DT, kind="ExternalInput")
    out = nc.dram_tensor("out", [IMGS, C, HW], IO_DT, kind="ExternalOutput")

    act_v = act.ap().rearrange("n (jc p) m -> n jc p m", p=P)
    gt_v = gt.ap().rearrange("(jc p) r -> jc p r", p=P)
    out_v = out.ap().rearrange("n (zc p) m -> n zc p m", p=P)

    ZCS = (1, 2, 3, 0)            # zc processing order (ascending chunk pairs)
    LOADS_FOR_ZC = {1: 2, 2: 3, 3: 4, 0: 4}
    QW = QT * PT                  # drain/store width: 1568 cols

    def quad_engine(q):           # strict alternation vector/scalar
        return "v" if q % 2 == 0 else "s"

    v_done_at = {}
    s_done_at = {}
    nv = ns = 0
    for q in range(NQ):
        if quad_engine(q) == "v":
            nv += 1
        else:
            ns += 1
        v_done_at[q] = nv
        s_done_at[q] = ns

    from contextlib import ExitStack
    with ExitStack() as ctx:
        a_sb = [ctx.enter_context(
            nc.sbuf_tensor(f"a_sb{i}", [P, NCHUNK * HW], IO_DT)).ap()
            for i in range(IMGS)]
        gt_sb = ctx.enter_context(
            nc.sbuf_tensor("gt_sb", [P, NCHUNK * C], IO_DT)).ap()
        o_sb = [[ctx.enter_context(
            nc.sbuf_tensor(f"o_sb{i}_{z}", [P, HW], IO_DT)).ap()
            for z in range(NCHUNK)] for i in range(IMGS)]
        ps = ctx.enter_context(
            nc.psum_tensor("ps", [P, 4096], mybir.dt.float32)).ap()

        s_gt = nc.alloc_semaphore("s_gt")
        s_ld = [nc.alloc_semaphore(f"s_ld{i}") for i in range(IMGS)]
        s_mm = nc.alloc_semaphore("s_mm")
        s_cv = nc.alloc_semaphore("s_cv")    # vector quad-drains done
        s_cs = nc.alloc_semaphore("s_cs")    # scalar quad-drains done
        s_st = nc.alloc_semaphore("s_st")
        all_sems = [s_gt, s_mm, s_cv, s_cs, s_st] + s_ld

        a3 = [a.rearrange("p (jc m) -> p jc m", jc=NCHUNK) for a in a_sb]
        gt3 = gt_sb.rearrange("p (jc r) -> p jc r", jc=NCHUNK)
        ps3 = ps.rearrange("p (s f) -> p s f", s=8)   # [128, 8, 512]

        def slot_ap(ti):          # matmul output: one 392-col bank region
            s = ti % 8
            return ps[:, s * 512:s * 512 + PT]

        def quad_ap(q):           # drain source: 4 slots x 392 cols
            s0 = (q % 2) * 4
            return ps3[:, s0:s0 + 4, :PT]

        def emit_drain(eng, inc_sem, q):
            img, rem = divmod(q, NCHUNK * 2)
            zci, half = divmod(rem, 2)
            zc = ZCS[zci]
            eng.wait_ge(s_mm, QT * (q + 1))
            dst = o_sb[img][zc][:, half * QW:(half + 1) * QW]
            if inc_sem is s_cv:
                eng.tensor_copy(dst, quad_ap(q)).then_inc(inc_sem, 1)
            else:
                eng.activation(dst, quad_ap(q),
                               mybir.ActivationFunctionType.Copy,
                               ).then_inc(inc_sem, 1)

        with nc.Block("clears") as blk:

            @blk.sync
            def _(sync):
                for s in all_sems:
                    sync.sem_clear(s)

        with nc.Block("main") as blk:

            @blk.gpsimd
            def _(g):
                # SWDGE ring: gt first (FIFO -> lands before the big loads)
                for jc in range(NCHUNK):
                    g.dma_start(gt_sb[:, jc * C:(jc + 1) * C], gt_v[jc]
                                ).then_inc(s_gt, 16)
                for img in range(IMGS):
                    for jc in range(NCHUNK):
                        g.dma_start(a3[img][:, jc], act_v[img, jc]
                                    ).then_inc(s_ld[img], 16)

            @blk.scalar
            def _(sc):
                for q in range(NQ):
                    if quad_engine(q) == "s":
                        emit_drain(sc, s_cs, q)

            @blk.vector
            def _(v):
                for q in range(NQ):
                    if quad_engine(q) == "v":
                        emit_drain(v, s_cv, q)

            @blk.tensor
            def _(t):
                t.wait_ge(s_gt, 16 * NCHUNK)
                # HAM warmup on gt data into slot 7 (reset by tile 7's start)
                for _i in range(N_WARMUP):
                    t.matmul(ps[:, 7 * 512:7 * 512 + PT], gt3[:, 0:2, :P],
                             gt3[:, 0:2, :PT], start=True, stop=True,
                             perf_mode=mybir.MatmulPerfMode.DoubleRow,
                             skip_group_check=True)
                ti = 0
                for img in range(IMGS):
                    for zci, zc in enumerate(ZCS):
                        t.wait_ge(s_ld[img], 16 * LOADS_FOR_ZC[zc])
                        for pt in range(NPT):
                            if ti % QT == 0 and ti >= 8:
                                q = (ti - 8) // QT
                                if quad_engine(q) == "v":
                                    t.wait_ge(s_cv, v_done_at[q])
                                else:
                                    t.wait_ge(s_cs, s_done_at[q])
                            po = slot_ap(ti)
                            msl = slice(pt * PT, (pt + 1) * PT)
                            if zc >= 1:
                                t.matmul(
                                    po, gt3[:, zc - 1:zc + 1, zc * P:(zc + 1) * P],
                                    a3[img][:, zc - 1:zc + 1, msl],
                                    start=True, stop=True,
                                    perf_mode=mybir.MatmulPerfMode.DoubleRow,
                                ).then_inc(s_mm, 1)
                            else:
                                t.matmul(po, gt3[:, 3, 0:P],
                                         a3[img][:, 3, msl],
                                         start=True, stop=False)
                                t.matmul(po, gt3[:, 0, 0:P],
                                         a3[img][:, 0, msl],
                                         start=False, stop=True,
                                         ).then_inc(s_mm, 1)
                            ti += 1

            @blk.sync
            def _(sync):
                for q in range(NQ):
                    img, rem = divmod(q, NCHUNK * 2)
                    zci, half = divmod(rem, 2)
                    zc = ZCS[zci]
                    if quad_engine(q) == "v":
                        sync.wait_ge(s_cv, v_done_at[q])
                    else:
                        sync.wait_ge(s_cs, s_done_at[q])
                    sync.dma_start(
                        out_v[img, zc, :, half * QW:(half + 1) * QW],
                        o_sb[img][zc][:, half * QW:(half + 1) * QW],
                    ).then_inc(s_st, 16)
                sync.wait_ge(s_st, 16 * NQ)

    nc.compile()
    return nc


def _make_gt(inhib_kernel: np.ndarray) -> np.ndarray:
    """Masked rotated circulant of the deconv correction, as fp8 lhsT.

    GTs[j, r] = h[(r - j) mod C] - delta[r==j], where h = roll(g, -ROT) and
    g = ifft(1/fft(k)); entries with chunk distance (r//P - j//P) mod 4 > 1
    are dropped (never touched by the kept matmuls).
    """
    k = np.asarray(inhib_kernel, dtype=np.float64)
    g = np.real(np.fft.ifft(1.0 / np.fft.fft(k)))
    h = np.roll(g, -ROT)
    r = np.arange(C)
    t = (r[None, :] - r[:, None]) % C          # [j, r]
    gts = h[t] - np.eye(C)
    d = ((r[None, :] // P) - (r[:, None] // P)) % NCHUNK
    gts *= (d <= 1)
    return np.ascontiguousarray(gts.astype(IO_NP))


def _prep_in_maps(acts_f32: np.ndarray, gt_np: np.ndarray):
    """Quantize activations to fp8 and shard per core."""
    acts8 = acts_f32.reshape(N, C, HW).astype(IO_NP)
    return [
        {"act": np.ascontiguousarray(acts8[c * IMGS:(c + 1) * IMGS]),
         "gt": gt_np}
        for c in range(N_CORES)
    ], acts8


def kernel(activations, inhib_kernel):
    acts = np.asarray(activations, dtype=np.float32)
    assert acts.shape == (N, C, H, W), acts.shape
    gt_np = _make_gt(np.asarray(inhib_kernel))

    if "nc" not in _CACHE:
        _CACHE["nc"] = _build_nc()
    nc = _CACHE["nc"]

    in_maps, acts8 = _prep_in_maps(acts, gt_np)
    res = run_bass_kernel_spmd(nc, in_maps, core_ids=list(range(N_CORES)))
    c_out = np.concatenate([r["out"] for r in res.results], axis=0)
    # z = x + c in the rotated frame (exact fp32 identity), then un-rotate
    z = acts.reshape(N, C, HW) + c_out.astype(np.float32)
    y = z[:, (np.arange(C) - ROT) % C, :]
    return np.ascontiguousarray(y.reshape(N, C, H, W))


# revision 8
# speedup vs baseline: 1.0958x; 1.0958x over previous
"""ConvergedInhibition TRN2 kernel (fp8 correction-matmul version).

The reference computes, per pixel (n,h,w), an FFT deconvolution along the
channel axis: y = ifft(fft(x)/fft(k)).real. Since k is fixed, this is a
circular convolution with g = ifft(1/fft(k)): a dense CxC circulant matmul
applied to every pixel, data-parallel over 32 images across 8 cores.

This version exploits the structure y = x + c where c = (G - I) x is a small
correction (||c|| ~ 0.14 ||y||): the device computes only the correction from
fp8(e4m3)-quantized activations and stores it as fp8, halving HBM traffic in
both directions (the DMA roofline). The exact fp32 identity term is added
back on the host during unsharding, so quantization noise only enters scaled
by the correction magnitude (measured total rel err ~8e-3 vs 2e-2 budget).

Rotated frame: z[r] = y[(r+ROT) mod C] aligns the deconv impulse response h
(one-sided, support ~[0,224)) to the diagonal. Keeping chunk distances
d=(zc-jc) mod 4 in {0,1} covers t in [0, 128+q] per output row q (trunc err
~2e-3). For zc>=1 the two kept input chunks are adjacent in SBUF, so each
output tile is ONE fp8 DoubleRow matmul (K=256 at 2x PE rate, 392cyc). zc=0
wraps (jc=3,0) and uses two plain fp8 matmuls instead.

Engine layout (per core): gpsimd issues gt then the 16 act loads on the
SWDGE ring (FIFO keeps gt ahead of the big loads); sync issues the 32 output
stores; vector and scalar alternate 4-tile (1568-col) PSUM->fp8 quad-drains;
tensor runs 160 matmuls at a measured 166ns/tile (LDWEIGHTS prefetch
overlaps matmuls via the PE reorder window). PSUM is one 8-bank tensor; the
drain of quad Q gates tensor's reuse of those 4 banks at quad Q+2.
"""

import numpy as np
import ml_dtypes

import concourse.bass as bass  # noqa: F401  (registers bass types)
import concourse.mybir as mybir
from concourse import bacc
from concourse.bass_utils import run_bass_kernel_spmd

N_CORES = 8
N, C, H, W = 32, 512, 56, 56
HW = H * W                      # 3136
IMGS = N // N_CORES             # 4 images per core
P = 128                         # partitions
NCHUNK = C // P                 # 4
PT = 392                        # pixel tile (free dim), 3136 = 8*392
NPT = HW // PT                  # 8
ROT = 288                       # rotation aligning h's one-sided support
IO_DT = mybir.dt.float8e4
IO_NP = ml_dtypes.float8_e4m3   # matches TRN FP8_EXP4 semantics
N_WARMUP = 14                   # HAM clock-gate warmup matmuls
QT = 4                          # tiles per drain quad
NQ = IMGS * NCHUNK * NPT // QT  # 32 quads

_CACHE = {}


def _build_nc():
    """Raw bacc engine programs with explicit semaphores."""
    nc = bacc.Bacc("TRN2", target_bir_lowering=False, debug=False,
                   num_devices=N_CORES)
    act = nc.dram_tensor("act", [IMGS, C, HW], IO_DT, kind="ExternalInput")
    gt = nc.dram_tensor("gt", [C, C], IO_DT, kind="ExternalInput")
    out = nc.dram_tensor("out", [IMGS, C, HW], IO_DT, kind="ExternalOutput")

    act_v = act.ap().rearrange("n (jc p) m -> n jc p m", p=P)
    gt_v = gt.ap().rearrange("(jc p) r -> jc p r", p=P)
    out_v = out.ap().rearrange("n (zc p) m -> n zc p m", p=P)

    ZCS = (1, 2, 3, 0)            # zc processing order (ascending chunk pairs)
    LOADS_FOR_ZC = {1: 2, 2: 3, 3: 4, 0: 4}
    PW = 2 * PT                   # drain width: 784 cols (2 tiles)
    NP_ = IMGS * NCHUNK * NPT // 2  # 64 pairs

    def pair_engine(q):           # strict alternation vector/scalar
        return "v" if q % 2 == 0 else "s"

    v_done_at = {}
    s_done_at = {}
    nv = ns = 0
    for q in range(NP_):
        if pair_engine(q) == "v":
            nv += 1
        else:
            ns += 1
        v_done_at[q] = nv
        s_done_at[q] = ns

    from contextlib import ExitStack
    with ExitStack() as ctx:
        a_sb = [ctx.enter_context(
            nc.sbuf_tensor(f"a_sb{i}", [P, NCHUNK * HW], IO_DT)).ap()
            for i in range(IMGS)]
        gt_sb = ctx.enter_context(
            nc.sbuf_tensor("gt_sb", [P, NCHUNK * C], IO_DT)).ap()
        o_sb = [[ctx.enter_context(
            nc.sbuf_tensor(f"o_sb{i}_{z}", [P, HW], IO_DT)).ap()
            for z in range(NCHUNK)] for i in range(IMGS)]
        ps = ctx.enter_context(
            nc.psum_tensor("ps", [P, 4096], mybir.dt.float32)).ap()

        s_gt = nc.alloc_semaphore("s_gt")
        s_ld = [nc.alloc_semaphore(f"s_ld{i}") for i in range(IMGS)]
        s_mm = nc.alloc_semaphore("s_mm")
        s_cv = nc.alloc_semaphore("s_cv")    # vector quad-drains done
        s_cs = nc.alloc_semaphore("s_cs")    # scalar quad-drains done
        s_st = nc.alloc_semaphore("s_st")
        all_sems = [s_gt, s_mm, s_cv, s_cs, s_st] + s_ld

        a3 = [a.rearrange("p (jc m) -> p jc m", jc=NCHUNK) for a in a_sb]
        gt3 = gt_sb.rearrange("p (jc r) -> p jc r", jc=NCHUNK)
        ps3 = ps.rearrange("p (s f) -> p s f", s=8)   # [128, 8, 512]

        def slot_ap(ti):          # matmul output: one 392-col bank region
            s = ti % 8
            return ps[:, s * 512:s * 512 + PT]

        def pair_ap(q):           # drain source: 2 slots x 392 cols
            s0 = (q % 4) * 2
            return ps3[:, s0:s0 + 2, :PT]

        def emit_drain(eng, inc_sem, q):
            img, rem = divmod(q, NCHUNK * 4)
            zci, lp = divmod(rem, 4)
            zc = ZCS[zci]
            eng.wait_ge(s_mm, 2 * (q + 1))
            dst = o_sb[img][zc][:, lp * PW:(lp + 1) * PW]
            if inc_sem is s_cv:
                eng.tensor_copy(dst, pair_ap(q)).then_inc(inc_sem, 1)
            else:
                eng.activation(dst, pair_ap(q),
                               mybir.ActivationFunctionType.Copy,
                               ).then_inc(inc_sem, 1)

        with nc.Block("clears") as blk:

            @blk.sync
            def _(sync):
                for s in all_sems:
                    sync.sem_clear(s)

        with nc.Block("main") as blk:

            @blk.gpsimd
            def _(g):
                # SWDGE ring: all activation loads (gt rides the sync ring)
                for img in range(IMGS):
                    for jc in range(NCHUNK):
                        g.dma_start(a3[img][:, jc], act_v[img, jc]
                                    ).then_inc(s_ld[img], 16)

            @blk.scalar
            def _(sc):
                for q in range(NP_):
                    if pair_engine(q) == "s":
                        emit_drain(sc, s_cs, q)

            @blk.vector
            def _(v):
                for q in range(NP_):
                    if pair_engine(q) == "v":
                        emit_drain(v, s_cv, q)

            @blk.tensor
            def _(t):
                t.wait_ge(s_gt, 16 * NCHUNK)
                ti = 0
                for img in range(IMGS):
                    for zci, zc in enumerate(ZCS):
                        t.wait_ge(s_ld[img], 16 * LOADS_FOR_ZC[zc])
                        for pt in range(NPT):
                            if ti % 2 == 0 and ti >= 8:
                                q = (ti - 8) // 2
                                if pair_engine(q) == "v":
                                    t.wait_ge(s_cv, v_done_at[q])
                                else:
                                    t.wait_ge(s_cs, s_done_at[q])
                            po = slot_ap(ti)
                            msl = slice(pt * PT, (pt + 1) * PT)
                            if zc >= 1:
                                t.matmul(
                                    po, gt3[:, zc - 1:zc + 1, zc * P:(zc + 1) * P],
                                    a3[img][:, zc - 1:zc + 1, msl],
                                    start=True, stop=True,
                                    perf_mode=mybir.MatmulPerfMode.DoubleRow,
                                ).then_inc(s_mm, 1)
                            else:
                                t.matmul(po, gt3[:, 3, 0:P],
                                         a3[img][:, 3, msl],
                                         start=True, stop=False)
                                t.matmul(po, gt3[:, 0, 0:P],
                                         a3[img][:, 0, msl],
                                         start=False, stop=True,
                                         ).then_inc(s_mm, 1)
                            ti += 1

            @blk.sync
            def _(sync):
                # gt loads first (HWDGE ring is otherwise idle early)
                for jc in range(NCHUNK):
                    sync.dma_start(gt_sb[:, jc * C:(jc + 1) * C], gt_v[jc]
                                   ).then_inc(s_gt, 16)
                n_store = 0
                for q2 in range(NP_ // 2):   # store per 2 pairs (1568 cols)
                    img, rem = divmod(q2, NCHUNK * 2)
                    zci, half = divmod(rem, 2)
                    zc = ZCS[zci]
                    for q in (2 * q2, 2 * q2 + 1):
                        if pair_engine(q) == "v":
                            sync.wait_ge(s_cv, v_done_at[q])
                        else:
                            sync.wait_ge(s_cs, s_done_at[q])
                    sync.dma_start(
                        out_v[img, zc, :, half * 2 * PW:(half + 1) * 2 * PW],
                        o_sb[img][zc][:, half * 2 * PW:(half + 1) * 2 * PW],
                    ).then_inc(s_st, 16)
                    n_store += 1
                sync.wait_ge(s_st, 16 * n_store)

    nc.compile()
    return nc


def _make_gt(inhib_kernel: np.ndarray) -> np.ndarray:
    """Masked rotated circulant of the deconv correction, as fp8 lhsT.

    GTs[j, r] = h[(r - j) mod C] - delta[r==j], where h = roll(g, -ROT) and
    g = ifft(1/fft(k)); entries with chunk distance (r//P - j//P) mod 4 > 1
    are dropped (never touched by the kept matmuls).
    """
    k = np.asarray(inhib_kernel, dtype=np.float64)
    g = np.real(np.fft.ifft(1.0 / np.fft.fft(k)))
    h = np.roll(g, -ROT)
    r = np.arange(C)
    t = (r[None, :] - r[:, None]) % C          # [j, r]
    gts = h[t] - np.eye(C)
    d = ((r[None, :] // P) - (r[:, None] // P)) % NCHUNK
    gts *= (d <= 1)
    return np.ascontiguousarray(gts.astype(IO_NP))


def _prep_in_maps(acts_f32: np.ndarray, gt_np: np.ndarray):
    """Quantize activations to fp8 and shard per core."""
    acts8 = acts_f32.reshape(N, C, HW).astype(IO_NP)
    return [
        {"act": np.ascontiguousarray(acts8[c * IMGS:(c + 1) * IMGS]),
         "gt": gt_np}
        for c in range(N_CORES)
    ], acts8


def kernel(activations, inhib_kernel):
    acts = np.asarray(activations, dtype=np.float32)
    assert acts.shape == (N, C, H, W), acts.shape
    gt_np = _make_gt(np.asarray(inhib_kernel))

    if "nc" not in _CACHE:
        _CACHE["nc"] = _build_nc()
    nc = _CACHE["nc"]

    in_maps, acts8 = _prep_in_maps(acts, gt_np)
    res = run_bass_kernel_spmd(nc, in_maps, core_ids=list(range(N_CORES)))
    c_out = np.concatenate([r["out"] for r in res.results], axis=0)
    # z = x + c in the rotated frame (exact fp32 identity), then un-rotate
    z = acts.reshape(N, C, HW) + c_out.astype(np.float32)
    y = z[:, (np.arange(C) - ROT) % C, :]
    return np.ascontiguousarray(y.reshape(N, C, H, W))


# revision 14
# speedup vs baseline: 1.2987x; 1.1853x over previous
"""ConvergedInhibition TRN2 kernel (fp8 correction-matmul version).

The reference computes, per pixel (n,h,w), an FFT deconvolution along the
channel axis: y = ifft(fft(x)/fft(k)).real. Since k is fixed, this is a
circular convolution with g = ifft(1/fft(k)): a dense CxC circulant matmul
applied to every pixel, data-parallel over 32 images across 8 cores.

This version exploits the structure y = x + c where c = (G - I) x is a small
correction (||c|| ~ 0.14 ||y||): the device computes only the correction from
fp8(e4m3)-quantized activations and stores it as fp8, halving HBM traffic in
both directions (the DMA roofline). The exact fp32 identity term is added
back on the host during unsharding, so quantization noise only enters scaled
by the correction magnitude (measured total rel err ~8e-3 vs 2e-2 budget).

Rotated frame: z[r] = y[(r+ROT) mod C] aligns the deconv impulse response h
(one-sided, support ~[0,224)) to the diagonal. Keeping chunk distances
d=(zc-jc) mod 4 in {0,1} covers t in [0, 128+q] per output row q (trunc err
~2e-3). For zc>=1 the two kept input chunks are adjacent in SBUF, so each
output tile is ONE fp8 DoubleRow matmul (K=256 at 2x PE rate, 392cyc). zc=0
wraps (jc=3,0) and uses two plain fp8 matmuls instead.

Engine layout (per core): gpsimd issues gt then the 16 act loads on the
SWDGE ring (FIFO keeps gt ahead of the big loads); sync issues the 32 output
stores; vector and scalar alternate 4-tile (1568-col) PSUM->fp8 quad-drains;
tensor runs 160 matmuls at a measured 166ns/tile (LDWEIGHTS prefetch
overlaps matmuls via the PE reorder window). PSUM is one 8-bank tensor; the
drain of quad Q gates tensor's reuse of those 4 banks at quad Q+2.
"""

import numpy as np
import ml_dtypes

import concourse.bass as bass  # noqa: F401  (registers bass types)
import concourse.mybir as mybir
from concourse import bacc
from concourse.bass_utils import run_bass_kernel_spmd

N_CORES = 8
N, C, H, W = 32, 512, 56, 56
HW = H * W                      # 3136
IMGS = N // N_CORES             # 4 images per core
P = 128                         # partitions
NCHUNK = C // P                 # 4
PT = 392                        # pixel tile (free dim), 3136 = 8*392
NPT = HW // PT                  # 8
ROT = 288                       # rotation aligning h's one-sided support
IO_DT = mybir.dt.float8e4
IO_NP = ml_dtypes.float8_e4m3   # matches TRN FP8_EXP4 semantics
N_WARMUP = 14                   # HAM clock-gate warmup matmuls
QT = 4                          # tiles per drain quad
NQ = IMGS * NCHUNK * NPT // QT  # 32 quads

_CACHE = {}


def _build_nc():
    """Raw bacc engine programs with explicit semaphores."""
    nc = bacc.Bacc("TRN2", target_bir_lowering=False, debug=False,
                   num_devices=N_CORES)
    act = nc.dram_tensor("act", [IMGS, C, HW], IO_DT, kind="ExternalInput")
    gt = nc.dram_tensor("gt", [C, C], IO_DT, kind="ExternalInput")
    out = nc.dram_tensor("out", [IMGS, C, HW], IO_DT, kind="ExternalOutput")

    act_v = act.ap().rearrange("n (jc p) m -> n p jc m", p=P)
    gt_v = gt.ap().rearrange("(jc p) r -> p jc r", p=P)
    out_v = out.ap().rearrange("n (zc p) m -> n zc p m", p=P)

    ZCS = (1, 2, 3, 0)            # zc processing order (ascending chunk pairs)
    # act loads land in 2-chunk units: unit 1 = chunks {0,1}, unit 2 = {2,3}
    LOADS_FOR_ZC = {1: 1, 2: 2, 3: 2, 0: 2}
    PW = 2 * PT                   # drain width: 784 cols (2 tiles)
    NP_ = IMGS * NCHUNK * NPT // 2  # 64 pairs

    def pair_engine(q):           # strict alternation vector/scalar
        return "v" if q % 2 == 0 else "s"

    v_done_at = {}
    s_done_at = {}
    nv = ns = 0
    for q in range(NP_):
        if pair_engine(q) == "v":
            nv += 1
        else:
            ns += 1
        v_done_at[q] = nv
        s_done_at[q] = ns

    from contextlib import ExitStack
    with ExitStack() as ctx:
        a_sb = [ctx.enter_context(
            nc.sbuf_tensor(f"a_sb{i}", [P, NCHUNK * HW], IO_DT)).ap()
            for i in range(IMGS)]
        gt_sb = ctx.enter_context(
            nc.sbuf_tensor("gt_sb", [P, NCHUNK * C], IO_DT)).ap()
        o_sb = [[ctx.enter_context(
            nc.sbuf_tensor(f"o_sb{i}_{z}", [P, HW], IO_DT)).ap()
            for z in range(NCHUNK)] for i in range(IMGS)]
        ps = ctx.enter_context(
            nc.psum_tensor("ps", [P, 4096], mybir.dt.float32)).ap()

        s_gt = nc.alloc_semaphore("s_gt")
        s_ld = [nc.alloc_semaphore(f"s_ld{i}") for i in range(IMGS)]
        s_mm = nc.alloc_semaphore("s_mm")
        s_cv = nc.alloc_semaphore("s_cv")    # vector quad-drains done
        s_cs = nc.alloc_semaphore("s_cs")    # scalar quad-drains done
        s_st = nc.alloc_semaphore("s_st")
        all_sems = [s_gt, s_mm, s_cv, s_cs, s_st] + s_ld

        a3 = [a.rearrange("p (jc m) -> p jc m", jc=NCHUNK) for a in a_sb]
        gt3 = gt_sb.rearrange("p (jc r) -> p jc r", jc=NCHUNK)
        ps3 = ps.rearrange("p (s f) -> p s f", s=8)   # [128, 8, 512]

        def slot_ap(ti):          # matmul output: one 392-col bank region
            s = ti % 8
            return ps[:, s * 512:s * 512 + PT]

        def pair_ap(q):           # drain source: 2 slots x 392 cols
            s0 = (q % 4) * 2
            return ps3[:, s0:s0 + 2, :PT]

        def emit_drain(eng, inc_sem, q):
            img, rem = divmod(q, NCHUNK * 4)
            zci, lp = divmod(rem, 4)
            zc = ZCS[zci]
            eng.wait_ge(s_mm, 2 * (q + 1))
            dst = o_sb[img][zc][:, lp * PW:(lp + 1) * PW]
            if inc_sem is s_cv:
                eng.tensor_copy(dst, pair_ap(q)).then_inc(inc_sem, 1)
            else:
                eng.activation(dst, pair_ap(q),
                               mybir.ActivationFunctionType.Copy,
                               ).then_inc(inc_sem, 1)

        with nc.Block("clears") as blk:

            @blk.sync
            def _(sync):
                for s in all_sems:
                    sync.sem_clear(s)

        with nc.Block("main") as blk:

            @blk.gpsimd
            def _(g):
                # SWDGE ring: gt first as ONE combined DMA (FIFO on the queue
                # guarantees it completes before the big act loads start),
                # then the act loads in 2-chunk units.
                g.dma_start(gt3, gt_v).then_inc(s_gt, 16)
                for img in range(IMGS):
                    for u in range(2):
                        g.dma_start(a3[img][:, 2 * u:2 * u + 2],
                                    act_v[img, :, 2 * u:2 * u + 2]
                                    ).then_inc(s_ld[img], 16)

            @blk.scalar
            def _(sc):
                for q in range(NP_):
                    if pair_engine(q) == "s":
                        emit_drain(sc, s_cs, q)

            @blk.vector
            def _(v):
                for q in range(NP_):
                    if pair_engine(q) == "v":
                        emit_drain(v, s_cv, q)

            @blk.tensor
            def _(t):
                t.wait_ge(s_gt, 16)
                ti = 0
                for img in range(IMGS):
                    for zci, zc in enumerate(ZCS):
                        t.wait_ge(s_ld[img], 16 * LOADS_FOR_ZC[zc])
                        for pt in range(NPT):
                            if ti % 2 == 0 and ti >= 8:
                                q = (ti - 8) // 2
                                if pair_engine(q) == "v":
                                    t.wait_ge(s_cv, v_done_at[q])
                                else:
                                    t.wait_ge(s_cs, s_done_at[q])
                            po = slot_ap(ti)
                            msl = slice(pt * PT, (pt + 1) * PT)
                            if zc >= 1:
                                t.matmul(
                                    po, gt3[:, zc - 1:zc + 1, zc * P:(zc + 1) * P],
                                    a3[img][:, zc - 1:zc + 1, msl],
                                    start=True, stop=True,
                                    perf_mode=mybir.MatmulPerfMode.DoubleRow,
                                ).then_inc(s_mm, 1)
                            else:
                                t.matmul(po, gt3[:, 3, 0:P],
                                         a3[img][:, 3, msl],
                                         start=True, stop=False)
                                t.matmul(po, gt3[:, 0, 0:P],
                                         a3[img][:, 0, msl],
                                         start=False, stop=True,
                                         ).then_inc(s_mm, 1)
                            ti += 1

            @blk.sync
            def _(sync):
                n_store = 0
                for q2 in range(NP_ // 2):   # store per 2 pairs (1568 cols)
                    img, rem = divmod(q2, NCHUNK * 2)
                    zci, half = divmod(rem, 2)
                    zc = ZCS[zci]
                    for q in (2 * q2, 2 * q2 + 1):
                        if pair_engine(q) == "v":
                            sync.wait_ge(s_cv, v_done_at[q])
                        else:
                            sync.wait_ge(s_cs, s_done_at[q])
                    sync.dma_start(
                        out_v[img, zc, :, half * 2 * PW:(half + 1) * 2 * PW],
                        o_sb[img][zc][:, half * 2 * PW:(half + 1) * 2 * PW],
                    ).then_inc(s_st, 16)
                    n_store += 1
                sync.wait_ge(s_st, 16 * n_store)

    nc.compile()
    return nc


def _make_gt(inhib_kernel: np.ndarray) -> np.ndarray:
    """Masked rotated circulant of the deconv correction, as fp8 lhsT.

    GTs[j, r] = h[(r - j) mod C] - delta[r==j], where h = roll(g, -ROT) and
    g = ifft(1/fft(k)); entries with chunk distance (r//P - j//P) mod 4 > 1
    are dropped (never touched by the kept matmuls).
    """
    k = np.asarray(inhib_kernel, dtype=np.float64)
    g = np.real(np.fft.ifft(1.0 / np.fft.fft(k)))
    h = np.roll(g, -ROT)
    r = np.arange(C)
    t = (r[None, :] - r[:, None]) % C          # [j, r]
    gts = h[t] - np.eye(C)
    d = ((r[None, :] // P) - (r[:, None] // P)) % NCHUNK
    gts *= (d <= 1)
    return np.ascontiguousarray(gts.astype(IO_NP))


def _prep_in_maps(acts_f32: np.ndarray, gt_np: np.ndarray):
    """Quantize activations to fp8 and shard per core."""
    acts8 = acts_f32.reshape(N, C, HW).astype(IO_NP)
    return [
        {"act": np.ascontiguousarray(acts8[c * IMGS:(c + 1) * IMGS]),
         "gt": gt_np}
        for c in range(N_CORES)
    ], acts8


def kernel(activations, inhib_kernel):
    acts = np.asarray(activations, dtype=np.float32)
    assert acts.shape == (N, C, H, W), acts.shape
    gt_np = _make_gt(np.asarray(inhib_kernel))

    if "nc" not in _CACHE:
        _CACHE["nc"] = _build_nc()
    nc = _CACHE["nc"]

    in_maps, acts8 = _prep_in_maps(acts, gt_np)
    res = run_bass_kernel_spmd(nc, in_maps, core_ids=list(range(N_CORES)))
    c_out = np.concatenate([r["out"] for r in res.results], axis=0)
    # z = x + c in the rotated frame (exact fp32 identity), then un-rotate
    z = acts.reshape(N, C, HW) + c_out.astype(np.float32)
    y = z[:, (np.arange(C) - ROT) % C, :]
    return np.ascontiguousarray(y.reshape(N, C, H, W))
